# revision 24
# baseline (speedup 1.0000x reference)
"""BinaryLinear (LLaMA-7B up-projection with sign-binarized weights) on 8 TRN2
NeuronCores — hybrid fp16 / fp8-DoubleRow precision.

Computes out = x @ sign(weight).T + bias for
  x      [4, 2048, 4096] fp16
  weight [11008, 4096]   fp16
  bias   [11008]         fp16
-> out   [4, 2048, 11008] fp16

Sharding: 2D tensor-parallel — features split in 2 halves of 5504 (= 43 tiles
of 128), tokens split in 4 quarters of 2048. Core c handles feature half
c % 2, token quarter c // 2. No collectives; the host gathers the 8 disjoint
output shards.

Precision split: sign(weight) in {-1,0,+1} is exact in fp8-e4m3, so fp8
matmul error comes only from quantizing x. Full-e4m3 x gives rel err
2.63e-2 (> the 2e-2 budget); keeping the first 1792 input channels in fp16
and quantizing the last 2304 to e4m3 scales the error by sqrt(2304/4096)
-> 1.976e-2 measured on HW (deterministic: fixed seed, fixed NEFF), while
the fp8 part runs in DoubleRow perf mode (two 128-deep k-tiles contracted
per instruction at fp8's 2x rate). Per-epoch matmul units: 14 fp16 + 9
DoubleRow = 23, vs 32 all-fp16 — a 1.39x PE-time reduction.

Per-core device kernel:
  - x shard resident in SBUF: 14 fp16 k-tiles [128ki, 2048t] + 9 fp8 pair
    tiles [128ki, 2ko, 2048t] (12.85MB total), DMA'd once.
  - Weights stream per feature tile: [128ki, 14ko, 128f] fp16 (448KB) +
    [128ki, 18ko, 128f] fp8 (288KB), host pre-swizzled, triple-buffered.
  - Phase A (first 2 feature tiles) runs k-outer with all 8 PSUM banks
    accumulating so matmuls start as soon as the first x chunk lands.
  - Phase B runs f-outer: per (feature tile f, token block t of 512),
    14 fp16 matmuls + 9 fp8 DoubleRow matmuls (lhsT [128k,2,128f], rhs
    [128k,2,512t]) accumulate into one PSUM bank; ScalarE applies
    per-partition bias + fp32->fp16 cast; one DMA per feature tile out.

sign(weight), the e4m3 quantization, layout swizzles, and the output gather
run on the host — layout prep, off the device critical path.
"""

import numpy as np

B, S, IN, OUT = 4, 2048, 4096, 11008
TOKENS = B * S  # 8192
NCORES = 8
FSPLIT = 2  # feature halves
TSPLIT = 4  # token quarters
F_SHARD = OUT // FSPLIT  # 5504
T_SHARD = TOKENS // TSPLIT  # 2048
P = 128
F_TILES = F_SHARD // P  # 43
K_TILES = IN // P  # 32
NK16 = 14  # fp16 k-tiles (input channels 0..NK16*128-1)
NPAIR = (K_TILES - NK16) // 2  # 9 fp8 DoubleRow pairs (remaining channels)
NK8 = 2 * NPAIR
NB = 512  # tokens per PSUM epoch
T_BLOCKS = T_SHARD // NB  # 4

_cached_nc = None


def _build_nc():
    import concourse.mybir as mybir
    import concourse.tile as tile
    from concourse import bacc

    nc = bacc.Bacc(
        "TRN2",
        target_bir_lowering=False,
        debug=False,
        enable_asserts=False,
    )

    DR = mybir.MatmulPerfMode.DoubleRow

    xt16 = nc.dram_tensor(
        "xt16", [P, NK16, T_SHARD], mybir.dt.float16, kind="ExternalInput"
    )
    xq8 = nc.dram_tensor(
        "xq8", [P, NK8, T_SHARD], mybir.dt.float8e4, kind="ExternalInput"
    )
    wt16 = nc.dram_tensor(
        "wt16", [F_TILES, P, NK16, P], mybir.dt.float16, kind="ExternalInput"
    )
    wq8 = nc.dram_tensor(
        "wq8", [F_TILES, P, NK8, P], mybir.dt.float8e4, kind="ExternalInput"
    )
    bias = nc.dram_tensor("bias", [P, F_TILES], mybir.dt.float32, kind="ExternalInput")
    out = nc.dram_tensor(
        "out", [F_SHARD, T_SHARD], mybir.dt.float16, kind="ExternalOutput"
    )

    xt16_ap = xt16.ap()
    xq8_ap = xq8.ap()
    wt16_ap = wt16.ap()
    wq8_ap = wq8.ap()
    out_ap = out.ap()

    # First WARM_F feature tiles run k-outer across all 8 PSUM banks so the
    # PE starts as soon as the first per-ko x chunk lands, overlapping the
    # x-shard load instead of idling behind it.
    WARM_F = 2

    with tile.TileContext(nc) as tc:
        with (
            tc.tile_pool(name="x", bufs=1) as xp,
            tc.tile_pool(name="const", bufs=1) as cp,
            tc.tile_pool(name="wwarm", bufs=1) as wwp,
            tc.tile_pool(name="w16", bufs=3) as wp16,
            tc.tile_pool(name="w8", bufs=3) as wp8,
            tc.tile_pool(name="o", bufs=3) as op,
            tc.tile_pool(name="ps", bufs=8, space="PSUM") as pp,
        ):
            # PE warm-up: ScalarE zeroes a scratch tile, then dummy matmuls
            # keep the PE busy until the first real matmuls' data lands
            # (~13.5us: DMA issue + transfer latency), so the HAM
            # clock-gate is at 8/8 when real matmuls start. Starting the
            # dummies earlier (e.g. GpSimd memset) just ends them before
            # the data arrives and the PE idle-stalls — data arrival, not
            # the memzero, is the binding constraint.
            scratch = cp.tile([P, NB], mybir.dt.float16)
            nc.scalar.memzero(scratch[:])
            ps_wu = pp.tile([P, NB], mybir.dt.float32, name="ps_wu", tag="ps")
            # 22 tuned empirically: the warmups end right as the first real
            # matmuls' data lands (~13.5us; the warm phase is DMA-arrival-
            # bound, so starting real matmuls earlier just converts dummy
            # time into idle gaps — measured 18 dummies: same total, with a
            # stall at +17.5us); fewer lets the PE idle-stall into a HAM
            # re-throttle, more delays the real stream.
            for _ in range(22):
                nc.tensor.matmul(
                    ps_wu[:], scratch[:, :P], scratch[:], start=True, stop=True
                )

            # Warm-phase weights in small chunks so the first matmuls are
            # gated on ~130KB, not a full tile. fp16 chunks of up to 4
            # k-tiles, fp8 chunks of up to 2 pairs (4 k-tiles).
            WKC = 4
            W16_CHUNKS = [
                (a, min(a + WKC, NK16)) for a in range(0, NK16, WKC)
            ]  # k-tile ranges
            W8_CHUNKS = [
                (a, min(a + 2, NPAIR)) for a in range(0, NPAIR, 2)
            ]  # pair ranges
            w_warm16 = [[None] * len(W16_CHUNKS) for _ in range(WARM_F)]
            w_warm8 = [[None] * len(W8_CHUNKS) for _ in range(WARM_F)]

            def _emit_warm_w16_one(c, f):
                a, b = W16_CHUNKS[c]
                w_sb = wwp.tile(
                    [P, b - a, P],
                    mybir.dt.float16,
                    name=f"ww16f{f}c{c}",
                    tag=f"ww16f{f}c{c}",
                )
                nc.sync.dma_start(w_sb[:], wt16_ap[f, :, a:b, :])
                w_warm16[f][c] = w_sb

            def _emit_warm_w16(c):
                for f in range(WARM_F):
                    _emit_warm_w16_one(c, f)

            def _emit_warm_w8(c):
                a, b = W8_CHUNKS[c]
                for f in range(WARM_F):
                    w_sb = wwp.tile(
                        [P, 2 * (b - a), P],
                        mybir.dt.float8e4,
                        name=f"ww8f{f}c{c}",
                        tag=f"ww8f{f}c{c}",
                    )
                    nc.sync.dma_start(w_sb[:], wq8_ap[f, :, 2 * a : 2 * b, :])
                    w_warm8[f][c] = w_sb

            # x shard as per-ko tiles so deps are chunk-granular; ko=0 is
            # further split into 4 per-token-block quarters so the first
            # matmul waits on 128KB of x, not 512KB.
            xs = [None] * NK16
            xs0q = [None] * T_BLOCKS
            xqs = [None] * NPAIR

            def _emit_x0q(t):
                x_sb = xp.tile([P, NB], mybir.dt.float16, name=f"x0q{t}", tag=f"x0q{t}")
                nc.sync.dma_start(x_sb[:], xt16_ap[:, 0, t * NB : (t + 1) * NB])
                xs0q[t] = x_sb

            def _emit_x(k):
                x_sb = xp.tile([P, T_SHARD], mybir.dt.float16, name=f"x{k}", tag=f"x{k}")
                nc.sync.dma_start(x_sb[:], xt16_ap[:, k, :])
                xs[k] = x_sb

            def _emit_xq(j):
                x_sb = xp.tile(
                    [P, 2, T_SHARD], mybir.dt.float8e4, name=f"xq{j}", tag=f"xq{j}"
                )
                nc.sync.dma_start(x_sb[:], xq8_ap[:, 2 * j : 2 * j + 2, :])
                xqs[j] = x_sb

            def xcol(k, lo, sw):
                if k == 0:
                    t, off = lo // NB, lo % NB
                    return xs0q[t][:, off : off + sw]
                return xs[k][:, lo : lo + sw]

            def x_slice(k, t):
                return xcol(k, t * NB, NB)

            def xq_slice(j, t):
                return xqs[j][:, :, t * NB : (t + 1) * NB]

            # Emission schedule: x tiles in warm-consumption order, with
            # each warm-w chunk issued ~4 x-emissions ahead of the first
            # warm unit that needs it (queues drain roughly in issue
            # order).
            # Critical first pair: the very first real matmul (k0, f0, t0)
            # needs only f0's first w chunk + the first x quarter — issue
            # those two back-to-back so the warm start isn't gated on f1's
            # chunk (DMA issues serialize at ~600ns each on Sync).
            _emit_warm_w16_one(0, 0)
            _emit_x0q(0)
            _emit_warm_w16_one(0, 1)

            x_emits = (
                [("x0q", t) for t in range(1, T_BLOCKS)]
                + [("x16", k) for k in range(1, NK16)]
                + [("xq", j) for j in range(NPAIR)]
            )

            def unit_to_xidx(u):
                # index into x_emits after which warm unit u's x is emitted
                return 0 if u == 0 else T_BLOCKS - 2 + u

            w_emits = sorted(
                [
                    (max(0, unit_to_xidx(a) - 4), "w16", c)
                    for c, (a, _) in enumerate(W16_CHUNKS)
                    if c > 0
                ]
                + [
                    (max(0, unit_to_xidx(NK16 + a) - 4), "w8", c)
                    for c, (a, _) in enumerate(W8_CHUNKS)
                ]
            )
            wi = 0
            for xi, (kind, v) in enumerate(x_emits):
                while wi < len(w_emits) and w_emits[wi][0] <= xi:
                    (_emit_warm_w16 if w_emits[wi][1] == "w16" else _emit_warm_w8)(
                        w_emits[wi][2]
                    )
                    wi += 1
                if kind == "x0q":
                    _emit_x0q(v)
                elif kind == "x16":
                    _emit_x(v)
                else:
                    _emit_xq(v)
            while wi < len(w_emits):
                (_emit_warm_w16 if w_emits[wi][1] == "w16" else _emit_warm_w8)(
                    w_emits[wi][2]
                )
                wi += 1
            bias_sb = cp.tile([P, F_TILES], mybir.dt.float32)
            nc.sync.dma_start(bias_sb[:], bias.ap()[:])

            # Phase A: k-outer warm start for f = 0..WARM_F-1. Units: NK16
            # fp16 k-tiles then NPAIR fp8 DoubleRow pairs.
            ps_warm = [
                [
                    pp.tile([P, NB], mybir.dt.float32, name="ps", tag="ps")
                    for _ in range(T_BLOCKS)
                ]
                for _ in range(WARM_F)
            ]
            for k in range(NK16):
                c = k // WKC
                r = k - W16_CHUNKS[c][0]
                for f in range(WARM_F):
                    for t in range(T_BLOCKS):
                        nc.tensor.matmul(
                            ps_warm[f][t][:],
                            w_warm16[f][c][:, r, :],
                            x_slice(k, t),
                            start=(k == 0),
                            stop=False,
                        )
            for j in range(NPAIR):
                c = j // 2
                r = j - W8_CHUNKS[c][0]
                for f in range(WARM_F):
                    for t in range(T_BLOCKS):
                        nc.tensor.matmul(
                            ps_warm[f][t][:],
                            w_warm8[f][c][:, 2 * r : 2 * r + 2, :],
                            xq_slice(j, t),
                            start=False,
                            stop=(j == NPAIR - 1),
                            perf_mode=DR,
                        )
            for f in range(WARM_F):
                o_sb = op.tile([P, T_SHARD], mybir.dt.float16)
                for t in range(T_BLOCKS):
                    nc.scalar.activation(
                        o_sb[:, t * NB : (t + 1) * NB],
                        ps_warm[f][t][:],
                        mybir.ActivationFunctionType.Identity,
                        bias=bias_sb[:, f : f + 1],
                    )
                nc.sync.dma_start(out_ap[f * P : (f + 1) * P, :], o_sb[:])

            # Phase B: f-outer steady state, x fully resident. The last f
            # tile is evicted per token block so the kernel tail is one
            # small DMA, not a 512KB one.
            for f in range(WARM_F, F_TILES):
                w16_sb = wp16.tile([P, NK16, P], mybir.dt.float16, tag="w16")
                nc.sync.dma_start(w16_sb[:], wt16_ap[f])
                w8_sb = wp8.tile([P, NK8, P], mybir.dt.float8e4, tag="w8")
                nc.sync.dma_start(w8_sb[:], wq8_ap[f])
                last = f == F_TILES - 1
                o_sb = op.tile([P, T_SHARD], mybir.dt.float16)
                for t in range(T_BLOCKS):
                    ps = pp.tile([P, NB], mybir.dt.float32)
                    for k in range(NK16):
                        nc.tensor.matmul(
                            ps[:],
                            w16_sb[:, k, :],
                            x_slice(k, t),
                            start=(k == 0),
                            stop=False,
                        )
                    for j in range(NPAIR):
                        nc.tensor.matmul(
                            ps[:],
                            w8_sb[:, 2 * j : 2 * j + 2, :],
                            xq_slice(j, t),
                            start=False,
                            stop=(j == NPAIR - 1),
                            perf_mode=DR,
                        )
                    nc.scalar.activation(
                        o_sb[:, t * NB : (t + 1) * NB],
                        ps[:],
                        mybir.ActivationFunctionType.Identity,
                        bias=bias_sb[:, f : f + 1],
                    )
                    if last:
                        nc.sync.dma_start(
                            out_ap[f * P : (f + 1) * P, t * NB : (t + 1) * NB],
                            o_sb[:, t * NB : (t + 1) * NB],
                        )
                if not last:
                    nc.sync.dma_start(out_ap[f * P : (f + 1) * P, :], o_sb[:])
    nc.compile()
    return nc


def _get_nc():
    global _cached_nc
    if _cached_nc is None:
        _cached_nc = _build_nc()
    return _cached_nc


_last_results = None  # BassKernelResults of the most recent run (for test harness)


def kernel(x, weight, bias, _trace=False, _trace_cores=None):
    global _last_results
    import ml_dtypes
    from concourse.bass_utils import run_bass_kernel_spmd

    x = np.asarray(x).astype(np.float16, copy=False)
    weight = np.asarray(weight)
    bias = np.asarray(bias)
    assert x.shape == (B, S, IN) and weight.shape == (OUT, IN) and bias.shape == (OUT,)

    nc = _get_nc()

    KCUT = NK16 * P  # 1792: channels below stay fp16, above go e4m3

    # xT [IN, TOKENS] -> per token-quarter fp16 [128ki, 14ko, 2048t] and
    # e4m3 [128ki, 18ko, 2048t]
    xt = x.reshape(TOKENS, IN).T  # [IN, TOKENS] (view)
    xt16_quarters = []
    xq8_quarters = []
    for i in range(TSPLIT):
        q = xt[:, i * T_SHARD : (i + 1) * T_SHARD]
        xt16_quarters.append(
            np.ascontiguousarray(
                q[:KCUT].reshape(NK16, P, T_SHARD).transpose(1, 0, 2)
            )
        )
        xq8_quarters.append(
            np.ascontiguousarray(
                q[KCUT:].reshape(NK8, P, T_SHARD).transpose(1, 0, 2)
            ).astype(ml_dtypes.float8_e4m3)
        )

    ws = np.sign(weight).astype(np.float16)  # [OUT, IN]
    bias_f32 = bias.astype(np.float32)
    # per feature-half: fp16 [43ft, 128ki, 14ko, 128f] + e4m3 [43ft, 128ki,
    # 18ko, 128f], swizzled so each f-tile is one contiguous per-partition run
    wt16_halves = []
    wq8_halves = []
    bias_halves = []
    for j in range(FSPLIT):
        wsj = ws[j * F_SHARD : (j + 1) * F_SHARD, :].T  # [IN, F_SHARD] (view)
        wt16_halves.append(
            np.ascontiguousarray(
                wsj[:KCUT].reshape(NK16, P, F_TILES, P).transpose(2, 1, 0, 3)
            )
        )
        wq8_halves.append(
            np.ascontiguousarray(
                wsj[KCUT:].reshape(NK8, P, F_TILES, P).transpose(2, 1, 0, 3)
            ).astype(ml_dtypes.float8_e4m3)
        )
        bias_halves.append(
            np.ascontiguousarray(
                bias_f32[j * F_SHARD : (j + 1) * F_SHARD].reshape(F_TILES, P).T
            )
        )

    in_maps = []
    for c in range(NCORES):
        j, i = c % FSPLIT, c // FSPLIT
        in_maps.append(
            {
                "xt16": xt16_quarters[i],
                "xq8": xq8_quarters[i],
                "wt16": wt16_halves[j],
                "wq8": wq8_halves[j],
                "bias": bias_halves[j],
            }
        )

    res = run_bass_kernel_spmd(
        nc,
        in_maps,
        core_ids=list(range(NCORES)),
        trace=_trace,
        trace_cores=_trace_cores,
    )
    _last_results = res

    full = np.empty((OUT, TOKENS), dtype=np.float16)
    for c in range(NCORES):
        j, i = c % FSPLIT, c // FSPLIT
        full[
            j * F_SHARD : (j + 1) * F_SHARD, i * T_SHARD : (i + 1) * T_SHARD
        ] = res.results[c]["out"]
    return np.ascontiguousarray(full.T).reshape(B, S, OUT)


# revision 26
# speedup vs baseline: 1.0034x; 1.0034x over previous
"""BinaryLinear (LLaMA-7B up-projection with sign-binarized weights) on 8 TRN2
NeuronCores — hybrid fp16 / fp8-DoubleRow precision.

Computes out = x @ sign(weight).T + bias for
  x      [4, 2048, 4096] fp16
  weight [11008, 4096]   fp16
  bias   [11008]         fp16
-> out   [4, 2048, 11008] fp16

Sharding: 2D tensor-parallel — features split in 2 halves of 5504 (= 43 tiles
of 128), tokens split in 4 quarters of 2048. Core c handles feature half
c % 2, token quarter c // 2. No collectives; the host gathers the 8 disjoint
output shards.

Precision split: sign(weight) in {-1,0,+1} is exact in fp8-e4m3, so fp8
matmul error comes only from quantizing x. Full-e4m3 x gives rel err
2.63e-2 (> the 2e-2 budget); keeping the first 1792 input channels in fp16
and quantizing the last 2304 to e4m3 scales the error by sqrt(2304/4096)
-> 1.976e-2 measured on HW (deterministic: fixed seed, fixed NEFF), while
the fp8 part runs in DoubleRow perf mode (two 128-deep k-tiles contracted
per instruction at fp8's 2x rate). Per-epoch matmul units: 14 fp16 + 9
DoubleRow = 23, vs 32 all-fp16 — a 1.39x PE-time reduction.

Per-core device kernel:
  - x shard resident in SBUF: 14 fp16 k-tiles [128ki, 2048t] + 9 fp8 pair
    tiles [128ki, 2ko, 2048t] (12.85MB total), DMA'd once.
  - Weights stream per feature tile: [128ki, 14ko, 128f] fp16 (448KB) +
    [128ki, 18ko, 128f] fp8 (288KB), host pre-swizzled, triple-buffered.
  - Phase A (first 2 feature tiles) runs k-outer with all 8 PSUM banks
    accumulating so matmuls start as soon as the first x chunk lands.
  - Phase B runs f-outer: per (feature tile f, token block t of 512),
    14 fp16 matmuls + 9 fp8 DoubleRow matmuls (lhsT [128k,2,128f], rhs
    [128k,2,512t]) accumulate into one PSUM bank; ScalarE applies
    per-partition bias + fp32->fp16 cast; one DMA per feature tile out.

sign(weight), the e4m3 quantization, layout swizzles, and the output gather
run on the host — layout prep, off the device critical path.
"""

import numpy as np

B, S, IN, OUT = 4, 2048, 4096, 11008
TOKENS = B * S  # 8192
NCORES = 8
FSPLIT = 2  # feature halves
TSPLIT = 4  # token quarters
F_SHARD = OUT // FSPLIT  # 5504
T_SHARD = TOKENS // TSPLIT  # 2048
P = 128
F_TILES = F_SHARD // P  # 43
K_TILES = IN // P  # 32
NK16 = 14  # fp16 k-tiles (input channels 0..NK16*128-1)
NPAIR = (K_TILES - NK16) // 2  # 9 fp8 DoubleRow pairs (remaining channels)
NK8 = 2 * NPAIR
NB = 512  # tokens per PSUM epoch
T_BLOCKS = T_SHARD // NB  # 4

_cached_nc = None


def _build_nc():
    import concourse.mybir as mybir
    import concourse.tile as tile
    from concourse import bacc

    nc = bacc.Bacc(
        "TRN2",
        target_bir_lowering=False,
        debug=False,
        enable_asserts=False,
    )

    DR = mybir.MatmulPerfMode.DoubleRow

    xt16 = nc.dram_tensor(
        "xt16", [P, NK16, T_SHARD], mybir.dt.float16, kind="ExternalInput"
    )
    xq8 = nc.dram_tensor(
        "xq8", [P, NK8, T_SHARD], mybir.dt.float8e4, kind="ExternalInput"
    )
    wt16 = nc.dram_tensor(
        "wt16", [F_TILES, P, NK16, P], mybir.dt.float16, kind="ExternalInput"
    )
    wq8 = nc.dram_tensor(
        "wq8", [F_TILES, P, NK8, P], mybir.dt.float8e4, kind="ExternalInput"
    )
    bias = nc.dram_tensor("bias", [P, F_TILES], mybir.dt.float32, kind="ExternalInput")
    out = nc.dram_tensor(
        "out", [F_SHARD, T_SHARD], mybir.dt.float16, kind="ExternalOutput"
    )

    xt16_ap = xt16.ap()
    xq8_ap = xq8.ap()
    wt16_ap = wt16.ap()
    wq8_ap = wq8.ap()
    out_ap = out.ap()

    # First WARM_F feature tiles run k-outer across all 8 PSUM banks so the
    # PE starts as soon as the first per-ko x chunk lands, overlapping the
    # x-shard load instead of idling behind it.
    WARM_F = 2

    with tile.TileContext(nc) as tc:
        with (
            tc.tile_pool(name="x", bufs=1) as xp,
            tc.tile_pool(name="const", bufs=1) as cp,
            tc.tile_pool(name="wwarm", bufs=1) as wwp,
            tc.tile_pool(name="w16", bufs=3) as wp16,
            tc.tile_pool(name="w8", bufs=3) as wp8,
            tc.tile_pool(name="o", bufs=3) as op,
            tc.tile_pool(name="ps", bufs=8, space="PSUM") as pp,
        ):
            # PE warm-up: ScalarE zeroes a scratch tile, then dummy matmuls
            # keep the PE busy until the first real matmuls' data lands
            # (~13.5us: DMA issue + transfer latency), so the HAM
            # clock-gate is at 8/8 when real matmuls start. Starting the
            # dummies earlier (e.g. GpSimd memset) just ends them before
            # the data arrives and the PE idle-stalls — data arrival, not
            # the memzero, is the binding constraint.
            scratch = cp.tile([P, NB], mybir.dt.float16)
            nc.scalar.memzero(scratch[:])
            ps_wu = pp.tile([P, NB], mybir.dt.float32, name="ps_wu", tag="ps")
            # 22 tuned empirically: the warmups end right as the first real
            # matmuls' data lands (~13.5us; the warm phase is DMA-arrival-
            # bound, so starting real matmuls earlier just converts dummy
            # time into idle gaps — measured 18 dummies: same total, with a
            # stall at +17.5us); fewer lets the PE idle-stall into a HAM
            # re-throttle, more delays the real stream.
            for _ in range(22):
                nc.tensor.matmul(
                    ps_wu[:], scratch[:, :P], scratch[:], start=True, stop=True
                )

            # Warm-phase weights in small chunks so the first matmuls are
            # gated on ~130KB, not a full tile. fp16 chunks of up to 4
            # k-tiles, fp8 chunks of up to 2 pairs (4 k-tiles).
            WKC = 4
            W16_CHUNKS = [
                (a, min(a + WKC, NK16)) for a in range(0, NK16, WKC)
            ]  # k-tile ranges
            W8_CHUNKS = [
                (a, min(a + 2, NPAIR)) for a in range(0, NPAIR, 2)
            ]  # pair ranges
            w_warm16 = [[None] * len(W16_CHUNKS) for _ in range(WARM_F)]
            w_warm8 = [[None] * len(W8_CHUNKS) for _ in range(WARM_F)]

            def _emit_warm_w16_one(c, f):
                a, b = W16_CHUNKS[c]
                w_sb = wwp.tile(
                    [P, b - a, P],
                    mybir.dt.float16,
                    name=f"ww16f{f}c{c}",
                    tag=f"ww16f{f}c{c}",
                )
                nc.sync.dma_start(w_sb[:], wt16_ap[f, :, a:b, :])
                w_warm16[f][c] = w_sb

            def _emit_warm_w16(c):
                for f in range(WARM_F):
                    _emit_warm_w16_one(c, f)

            def _emit_warm_w8(c):
                a, b = W8_CHUNKS[c]
                for f in range(WARM_F):
                    w_sb = wwp.tile(
                        [P, 2 * (b - a), P],
                        mybir.dt.float8e4,
                        name=f"ww8f{f}c{c}",
                        tag=f"ww8f{f}c{c}",
                    )
                    nc.sync.dma_start(w_sb[:], wq8_ap[f, :, 2 * a : 2 * b, :])
                    w_warm8[f][c] = w_sb

            # x shard as per-ko tiles so deps are chunk-granular; ko=0 is
            # further split into 4 per-token-block quarters so the first
            # matmul waits on 128KB of x, not 512KB.
            xs = [None] * NK16
            xs0q = [None] * T_BLOCKS
            xqs = [None] * NPAIR

            def _emit_x0q(t):
                x_sb = xp.tile([P, NB], mybir.dt.float16, name=f"x0q{t}", tag=f"x0q{t}")
                nc.sync.dma_start(x_sb[:], xt16_ap[:, 0, t * NB : (t + 1) * NB])
                xs0q[t] = x_sb

            def _emit_x(k):
                x_sb = xp.tile([P, T_SHARD], mybir.dt.float16, name=f"x{k}", tag=f"x{k}")
                nc.sync.dma_start(x_sb[:], xt16_ap[:, k, :])
                xs[k] = x_sb

            def _emit_xq(j):
                x_sb = xp.tile(
                    [P, 2, T_SHARD], mybir.dt.float8e4, name=f"xq{j}", tag=f"xq{j}"
                )
                nc.sync.dma_start(x_sb[:], xq8_ap[:, 2 * j : 2 * j + 2, :])
                xqs[j] = x_sb

            def xcol(k, lo, sw):
                if k == 0:
                    t, off = lo // NB, lo % NB
                    return xs0q[t][:, off : off + sw]
                return xs[k][:, lo : lo + sw]

            def x_slice(k, t):
                return xcol(k, t * NB, NB)

            def xq_slice(j, t):
                return xqs[j][:, :, t * NB : (t + 1) * NB]

            # Emission schedule: x tiles in warm-consumption order, with
            # each warm-w chunk issued ~4 x-emissions ahead of the first
            # warm unit that needs it (queues drain roughly in issue
            # order).
            x_emits = (
                [("x0q", t) for t in range(T_BLOCKS)]
                + [("x16", k) for k in range(1, NK16)]
                + [("xq", j) for j in range(NPAIR)]
            )

            def unit_to_xidx(u):
                # index into x_emits after which warm unit u's x is emitted
                return 0 if u == 0 else T_BLOCKS - 1 + u

            w_emits = sorted(
                [
                    (max(0, unit_to_xidx(a) - 4), "w16", c)
                    for c, (a, _) in enumerate(W16_CHUNKS)
                ]
                + [
                    (max(0, unit_to_xidx(NK16 + a) - 4), "w8", c)
                    for c, (a, _) in enumerate(W8_CHUNKS)
                ]
            )
            wi = 0
            for xi, (kind, v) in enumerate(x_emits):
                while wi < len(w_emits) and w_emits[wi][0] <= xi:
                    (_emit_warm_w16 if w_emits[wi][1] == "w16" else _emit_warm_w8)(
                        w_emits[wi][2]
                    )
                    wi += 1
                if kind == "x0q":
                    _emit_x0q(v)
                elif kind == "x16":
                    _emit_x(v)
                else:
                    _emit_xq(v)
            while wi < len(w_emits):
                (_emit_warm_w16 if w_emits[wi][1] == "w16" else _emit_warm_w8)(
                    w_emits[wi][2]
                )
                wi += 1
            bias_sb = cp.tile([P, F_TILES], mybir.dt.float32)
            nc.sync.dma_start(bias_sb[:], bias.ap()[:])

            # Phase A: k-outer warm start for f = 0..WARM_F-1. Units: NK16
            # fp16 k-tiles then NPAIR fp8 DoubleRow pairs.
            ps_warm = [
                [
                    pp.tile([P, NB], mybir.dt.float32, name="ps", tag="ps")
                    for _ in range(T_BLOCKS)
                ]
                for _ in range(WARM_F)
            ]
            for k in range(NK16):
                c = k // WKC
                r = k - W16_CHUNKS[c][0]
                for f in range(WARM_F):
                    for t in range(T_BLOCKS):
                        nc.tensor.matmul(
                            ps_warm[f][t][:],
                            w_warm16[f][c][:, r, :],
                            x_slice(k, t),
                            start=(k == 0),
                            stop=False,
                        )
            for j in range(NPAIR):
                c = j // 2
                r = j - W8_CHUNKS[c][0]
                for f in range(WARM_F):
                    for t in range(T_BLOCKS):
                        nc.tensor.matmul(
                            ps_warm[f][t][:],
                            w_warm8[f][c][:, 2 * r : 2 * r + 2, :],
                            xq_slice(j, t),
                            start=False,
                            stop=(j == NPAIR - 1),
                            perf_mode=DR,
                        )
            for f in range(WARM_F):
                o_sb = op.tile([P, T_SHARD], mybir.dt.float16)
                for t in range(T_BLOCKS):
                    nc.scalar.activation(
                        o_sb[:, t * NB : (t + 1) * NB],
                        ps_warm[f][t][:],
                        mybir.ActivationFunctionType.Identity,
                        bias=bias_sb[:, f : f + 1],
                    )
                nc.sync.dma_start(out_ap[f * P : (f + 1) * P, :], o_sb[:])

            # Phase B: f-outer steady state, x fully resident. The last f
            # tile is evicted per token block so the kernel tail is one
            # small DMA, not a 512KB one.
            for f in range(WARM_F, F_TILES):
                w16_sb = wp16.tile([P, NK16, P], mybir.dt.float16, tag="w16")
                nc.sync.dma_start(w16_sb[:], wt16_ap[f])
                w8_sb = wp8.tile([P, NK8, P], mybir.dt.float8e4, tag="w8")
                nc.sync.dma_start(w8_sb[:], wq8_ap[f])
                last = f == F_TILES - 1
                o_sb = op.tile([P, T_SHARD], mybir.dt.float16)
                for t in range(T_BLOCKS):
                    ps = pp.tile([P, NB], mybir.dt.float32)
                    for k in range(NK16):
                        nc.tensor.matmul(
                            ps[:],
                            w16_sb[:, k, :],
                            x_slice(k, t),
                            start=(k == 0),
                            stop=False,
                        )
                    for j in range(NPAIR):
                        nc.tensor.matmul(
                            ps[:],
                            w8_sb[:, 2 * j : 2 * j + 2, :],
                            xq_slice(j, t),
                            start=False,
                            stop=(j == NPAIR - 1),
                            perf_mode=DR,
                        )
                    nc.scalar.activation(
                        o_sb[:, t * NB : (t + 1) * NB],
                        ps[:],
                        mybir.ActivationFunctionType.Identity,
                        bias=bias_sb[:, f : f + 1],
                    )
                    if last:
                        nc.sync.dma_start(
                            out_ap[f * P : (f + 1) * P, t * NB : (t + 1) * NB],
                            o_sb[:, t * NB : (t + 1) * NB],
                        )
                if not last:
                    nc.sync.dma_start(out_ap[f * P : (f + 1) * P, :], o_sb[:])
    nc.compile()
    return nc


def _get_nc():
    global _cached_nc
    if _cached_nc is None:
        _cached_nc = _build_nc()
    return _cached_nc


_last_results = None  # BassKernelResults of the most recent run (for test harness)


def kernel(x, weight, bias, _trace=False, _trace_cores=None):
    global _last_results
    import ml_dtypes
    from concourse.bass_utils import run_bass_kernel_spmd

    x = np.asarray(x).astype(np.float16, copy=False)
    weight = np.asarray(weight)
    bias = np.asarray(bias)
    assert x.shape == (B, S, IN) and weight.shape == (OUT, IN) and bias.shape == (OUT,)

    nc = _get_nc()

    KCUT = NK16 * P  # 1792: channels below stay fp16, above go e4m3

    # xT [IN, TOKENS] -> per token-quarter fp16 [128ki, 14ko, 2048t] and
    # e4m3 [128ki, 18ko, 2048t]
    xt = x.reshape(TOKENS, IN).T  # [IN, TOKENS] (view)
    xt16_quarters = []
    xq8_quarters = []
    for i in range(TSPLIT):
        q = xt[:, i * T_SHARD : (i + 1) * T_SHARD]
        xt16_quarters.append(
            np.ascontiguousarray(
                q[:KCUT].reshape(NK16, P, T_SHARD).transpose(1, 0, 2)
            )
        )
        xq8_quarters.append(
            np.ascontiguousarray(
                q[KCUT:].reshape(NK8, P, T_SHARD).transpose(1, 0, 2)
            ).astype(ml_dtypes.float8_e4m3)
        )

    ws = np.sign(weight).astype(np.float16)  # [OUT, IN]
    bias_f32 = bias.astype(np.float32)
    # per feature-half: fp16 [43ft, 128ki, 14ko, 128f] + e4m3 [43ft, 128ki,
    # 18ko, 128f], swizzled so each f-tile is one contiguous per-partition run
    wt16_halves = []
    wq8_halves = []
    bias_halves = []
    for j in range(FSPLIT):
        wsj = ws[j * F_SHARD : (j + 1) * F_SHARD, :].T  # [IN, F_SHARD] (view)
        wt16_halves.append(
            np.ascontiguousarray(
                wsj[:KCUT].reshape(NK16, P, F_TILES, P).transpose(2, 1, 0, 3)
            )
        )
        wq8_halves.append(
            np.ascontiguousarray(
                wsj[KCUT:].reshape(NK8, P, F_TILES, P).transpose(2, 1, 0, 3)
            ).astype(ml_dtypes.float8_e4m3)
        )
        bias_halves.append(
            np.ascontiguousarray(
                bias_f32[j * F_SHARD : (j + 1) * F_SHARD].reshape(F_TILES, P).T
            )
        )

    in_maps = []
    for c in range(NCORES):
        j, i = c % FSPLIT, c // FSPLIT
        in_maps.append(
            {
                "xt16": xt16_quarters[i],
                "xq8": xq8_quarters[i],
                "wt16": wt16_halves[j],
                "wq8": wq8_halves[j],
                "bias": bias_halves[j],
            }
        )

    res = run_bass_kernel_spmd(
        nc,
        in_maps,
        core_ids=list(range(NCORES)),
        trace=_trace,
        trace_cores=_trace_cores,
    )
    _last_results = res

    full = np.empty((OUT, TOKENS), dtype=np.float16)
    for c in range(NCORES):
        j, i = c % FSPLIT, c // FSPLIT
        full[
            j * F_SHARD : (j + 1) * F_SHARD, i * T_SHARD : (i + 1) * T_SHARD
        ] = res.results[c]["out"]
    return np.ascontiguousarray(full.T).reshape(B, S, OUT)
